# revision 4
# baseline (speedup 1.0000x reference)
"""CTC loss on 8 Trainium2 NeuronCores — fwd/bwd meet-in-the-middle.

Same fp8 input pipeline as v2 (g[49,T,n] fp8_e4m3, 0.82MB/core, HWDGE
chunked loads, on-device 49->97 expansion). The serial alpha chain is
halved by running the forward recurrence (t=1..255) and the backward
beta recurrence (t=511..256) CONCURRENTLY as four 16-example chains,
meeting at t=255:  P = sum_s alpha_255(s) * beta_255(s).

Backward step (mask m at source instead of md2):
    beta_{t-1}(s) = ub(s) + ub(s+1) + vb(s+2),  ub = q_t*beta_t, vb = m*ub
so the expansion emits [q | md2*q] for t<256 and [q | m*q] for t>=256 —
zero extra SBUF or expansion work.  Scale: beta init carries 2^9 and the
meet multiply carries 2^77 (scalar_tensor_tensor) so fin stays inside
f32 normals; both constants are subtracted in the final log.
"""
import os
import sys
import numpy as np

for _p in ("/opt/trn_rl_repo", "/root/.axon_site/_ro/trn_rl_repo"):
    if os.path.isdir(_p) and _p not in sys.path:
        sys.path.insert(0, _p)

import ml_dtypes  # noqa: E402
import concourse.bass as bass  # noqa: E402
import concourse.bacc as bacc  # noqa: E402
import concourse.mybir as mybir  # noqa: E402
import concourse.tile as tile  # noqa: E402
from concourse.bass_utils import run_bass_kernel_spmd  # noqa: E402

BF = ml_dtypes.bfloat16
F32 = np.float32
FP8 = ml_dtypes.float8_e4m3

B, T, L, C = 256, 512, 48, 512
S = 2 * L + 1          # 97
NJ = L + 1             # 49
BLANK = C - 1
EPS = 1e-7
ZQ = 512.0
NCORES = 8
BPC = B // NCORES      # 32
TN = T * BPC
RESC = 32              # cs must stay < 2^64 for Ln
CHT = 16
NCHUNK = T // CHT
TH = 255               # meet point
BB = 9                 # beta-init boost bits (baked into aux e9596)
BA = 77                # meet-multiply boost bits

_C_W1 = 0
_C_W2 = 97
_C_ONES = 194
_C_SEL = 195
_C_E01 = 196
_C_WBC = 197
_C_MD2 = 294
_C_M = _C_MD2 + BPC            # 326: backward v-stream mask
_C_W1B = _C_M + BPC            # 358
_C_W2B = _C_W1B + S            # 455
_C_E95 = _C_W2B + S            # 552: beta init (2^BB at 95,96)
NCOL = _C_E95 + 1              # 553

_JMAP = np.where(np.arange(S) % 2 == 1, (np.arange(S) - 1) // 2, L)


def _host_we():
    we = np.zeros((NJ, S), dtype=F32)
    we[_JMAP, np.arange(S)] = 1.0
    return we.astype(FP8)


def _host_aux_const():
    aux = np.zeros((S, _C_MD2), dtype=F32)
    ss = np.arange(S)
    aux[ss, _C_W1 + ss] = 1.0
    aux[ss[1:] - 1, _C_W1 + ss[1:]] = 1.0
    aux[ss[2:] - 2, _C_W2 + ss[2:]] = 1.0
    aux[:, _C_ONES] = 1.0
    aux[S - 2:S, _C_SEL] = 1.0
    aux[0:2, _C_E01] = 1.0
    aux[0, _C_WBC:_C_WBC + S] = 1.0
    return aux.astype(BF)


def _host_aux_bwd():
    aux = np.zeros((S, NCOL - _C_W1B), dtype=F32)
    ss = np.arange(S)
    aux[ss, ss] = 1.0                            # W1b: k == s
    aux[ss[:-1] + 1, ss[:-1]] = 1.0              # W1b: k == s+1
    aux[ss[:-2] + 2, S + ss[:-2]] = 1.0          # W2b: k == s+2
    aux[S - 2:S, 2 * S] = float(2.0 ** BB)       # e9596 * 2^BB
    return aux.astype(BF)


def make_in_maps(y_true, y_pred):
    lab = np.asarray(y_true).astype(np.int64)
    y = np.asarray(y_pred, dtype=F32)

    idx = np.concatenate(
        [lab, np.full((B, 1), BLANK, np.int64)], axis=1)
    g = np.take_along_axis(y, idx[:, None, :], axis=2)
    g8 = ((g + EPS) * ZQ).astype(FP8)

    ext = np.full((B, S), BLANK, dtype=np.int64)
    ext[:, 1::2] = lab
    m = np.zeros((B, S), dtype=F32)
    m[:, 1] = 1.0
    odd = np.arange(3, S, 2)
    m[:, odd] = (ext[:, odd] != ext[:, odd - 2]).astype(F32)
    md2 = np.zeros((B, S), dtype=F32)
    md2[:, :S - 2] = m[:, 2:]

    we = _host_we()
    aux_const = _host_aux_const()
    aux_bwd = _host_aux_bwd()
    in_maps = []
    for core in range(NCORES):
        sl = slice(core * BPC, (core + 1) * BPC)
        gc = np.ascontiguousarray(
            g8[sl].transpose(2, 1, 0)).reshape(NJ, TN)
        g_in = np.concatenate([we, gc], axis=1)            # [NJ, S+TN]
        aux = np.concatenate(
            [aux_const, md2[sl].T.astype(BF), m[sl].T.astype(BF),
             aux_bwd], axis=1)                             # [S, NCOL]
        in_maps.append({"g": g_in, "aux": aux})
    return in_maps


def build_bass(n_ex=BPC, Tt=T):
    dtb = mybir.dt.bfloat16
    dtf = mybir.dt.float32
    dt8 = mybir.dt.float8e4
    tn = Tt * n_ex
    nresc_f = len(range(RESC, TH + 1, RESC))       # 7 (t=32..224)
    nresc_b = len(range(RESC, Tt - TH - 1, RESC))  # 7 (i=32..224)
    ncs = nresc_f + nresc_b + 1                    # 15

    nc = bacc.Bacc()
    g_d = nc.dram_tensor("g", [NJ, S + tn], dt8, kind="ExternalInput")
    aux_d = nc.dram_tensor("aux", [S, NCOL], dtb, kind="ExternalInput")
    loss_d = nc.dram_tensor("loss", [n_ex, 1], dtf, kind="ExternalOutput")

    with tile.TileContext(nc) as tc:
        with (
            tc.tile_pool(name="persist", bufs=1) as persist,
            tc.tile_pool(name="uv", bufs=2) as uv_pool,
        ):
            gt = persist.tile([NJ, S + tn], dt8, tag="gt")
            aux_t = persist.tile([S, NCOL], dtb, tag="aux")
            qr = persist.tile([S, Tt, 2, n_ex], dtb, tag="qr")
            z0t = persist.tile([S, n_ex], dtf, tag="z0t")
            zb0t = persist.tile([S, n_ex], dtf, tag="zb0t")
            bsb = persist.tile([S, n_ex], dtb, tag="bsb")
            prodt = persist.tile([S, n_ex], dtb, tag="prodt")
            cbuf = persist.tile([1, ncs, n_ex], dtf, tag="cbuf")
            logbuf = persist.tile([1, ncs, n_ex], dtf, tag="logbuf")
            rscale = persist.tile([1, n_ex], dtb, tag="rscale")
            llsum = persist.tile([1, n_ex], dtf, tag="llsum")
            lossb = persist.tile([1, n_ex], dtf, tag="lossb")

            qtr = tn // 4
            for qi in range(4):
                lo = S + qi * qtr if qi else 0
                hi = S + (qi + 1) * qtr
                eng = nc.sync if qi % 2 == 0 else nc.scalar
                eng.dma_start(gt[:, lo:hi], g_d[:, lo:hi])
            nc.scalar.dma_start(aux_t[:], aux_d[:])

            we_ap = gt[:, 0:S]
            w1 = aux_t[:, _C_W1:_C_W1 + S]
            w2 = aux_t[:, _C_W2:_C_W2 + S]
            w1b = aux_t[:, _C_W1B:_C_W1B + S]
            w2b = aux_t[:, _C_W2B:_C_W2B + S]
            ones_col = aux_t[:, _C_ONES:_C_ONES + 1]
            e01_col = aux_t[:, _C_E01:_C_E01 + 1]
            e95_col = aux_t[:, _C_E95:_C_E95 + 1]
            wbc_row = aux_t[0:1, _C_WBC:_C_WBC + S]
            md2_ap = aux_t[:, _C_MD2:_C_MD2 + n_ex]
            m_ap = aux_t[:, _C_M:_C_M + n_ex]

            # expansion: [q | md2*q] for t<=TH, [q | m*q] for t>TH
            with tc.tile_pool(name="exp", bufs=2, space="PSUM") as expP:
                for ci in range(NCHUNK):
                    tlo = ci * CHT
                    mask = md2_ap if tlo <= TH else m_ap
                    ex = expP.tile([S, CHT, n_ex], dtf, tag="ex",
                                   name=f"ex{ci}")
                    nc.tensor.matmul(
                        ex[:], we_ap,
                        gt[:, S + tlo * n_ex:S + (tlo + CHT) * n_ex],
                        start=True, stop=True)
                    nc.scalar.copy(qr[:, tlo:tlo + CHT, 0, :], ex[:])
                    nc.vector.tensor_tensor(
                        qr[:, tlo:tlo + CHT, 1, :], ex[:],
                        mask.unsqueeze(1).broadcast_to([S, CHT, n_ex]),
                        mybir.AluOpType.mult)

            nc.vector.tensor_tensor(
                z0t[:], qr[:, 0, 0, :],
                e01_col.broadcast_to([S, n_ex]), mybir.AluOpType.mult)
            nc.scalar.copy(zb0t[:], e95_col.broadcast_to([S, n_ex]))

            NG = 2
            gsz = n_ex // NG
            gsl = [slice(gg * gsz, (gg + 1) * gsz) for gg in range(NG)]
            # chains: 0,1 = fwd groups; 2,3 = bwd groups
            NCH = 4
            cg = [0, 1, 0, 1]
            with tc.tile_pool(name="zp", bufs=2, space="PSUM") as zP:
                yt = [[uv_pool.tile([S, 2, gsz], dtb, tag=f"y{c}{p}",
                                    name=f"y{c}{p}") for p in range(2)]
                      for c in range(NCH)]
                al_prev = [None] * NCH
                for i in range(Tt - TH):               # 0..256
                    for c in range(NCH):
                        fwd = c < 2
                        if fwd:
                            if i >= TH:
                                continue
                            t = i + 1                  # 1..255
                        else:
                            if i >= Tt - TH - 1:
                                continue
                            t = Tt - 1 - i             # 511..256
                        gs = gsl[cg[c]]
                        y = yt[c][i % 2]
                        if al_prev[c] is None:
                            src0 = z0t if fwd else zb0t
                            src_ap = src0[:, gs].unsqueeze(1)\
                                .broadcast_to([S, 2, gsz])
                        else:
                            src_ap = al_prev[c][:].unsqueeze(1)\
                                .broadcast_to([S, 2, gsz])
                        nc.vector.tensor_tensor(
                            y[:], src_ap, qr[:, t, :, gs],
                            mybir.AluOpType.mult)
                        resc_here = (
                            (fwd and t % RESC == 0 and t <= TH) or
                            (not fwd and i > 0 and i % RESC == 0))
                        if resc_here:
                            j = (t // RESC - 1) if fwd \
                                else (nresc_f + i // RESC - 1)
                            cs = zP.tile([1, gsz], dtf, tag=f"z{c}",
                                         name=f"cs_{c}_{i}")
                            nc.tensor.matmul(cs[:], ones_col, y[:, 0, :],
                                             start=True, stop=True)
                            with nc.allow_low_precision(
                                    reason="1/cs bcast via bf16 matmul; "
                                    "log uses f32 cs"):
                                nc.vector.reciprocal(rscale[:, gs], cs[:])
                            nc.scalar.copy(cbuf[:, j, gs], cs[:])
                            rb = zP.tile([S, gsz], dtf, tag=f"z{c}",
                                         name=f"rb_{c}_{i}")
                            nc.tensor.matmul(rb[:], wbc_row, rscale[:, gs],
                                             start=True, stop=True)
                            nc.vector.tensor_tensor(
                                y[:], y[:],
                                rb[:].unsqueeze(1).broadcast_to(
                                    [S, 2, gsz]),
                                mybir.AluOpType.mult)
                        al = zP.tile([S, gsz], dtf, tag=f"z{c}",
                                     name=f"al_{c}_{i}")
                        wa, wb_ = (w1, w2) if fwd else (w1b, w2b)
                        nc.tensor.matmul(al[:], wa, y[:, 0, :],
                                         start=True, stop=False)
                        nc.tensor.matmul(al[:], wb_, y[:, 1, :],
                                         start=False, stop=True)
                        al_prev[c] = al

                # meet: fin = sum_s (alpha*2^BA) * beta
                for g in range(NG):
                    gs = gsl[g]
                    nc.scalar.copy(bsb[:, gs], al_prev[2 + g][:])
                    nc.vector.scalar_tensor_tensor(
                        prodt[:, gs], al_prev[g][:], float(2.0 ** BA),
                        bsb[:, gs], mybir.AluOpType.mult,
                        mybir.AluOpType.mult)
                    fin = zP.tile([1, gsz], dtf, tag=f"z{g}",
                                  name=f"fin{g}")
                    nc.tensor.matmul(fin[:], ones_col, prodt[:, gs],
                                     start=True, stop=True)
                    nc.scalar.copy(cbuf[:, ncs - 1, gs], fin[:])

            nc.scalar.activation(logbuf[:], cbuf[:],
                                 mybir.ActivationFunctionType.Ln)
            nc.vector.tensor_reduce(
                llsum[:], logbuf[:].rearrange("p j b -> p b j"),
                mybir.AxisListType.X, mybir.AluOpType.add)
            bias = float(Tt * np.log(ZQ) + (BA + BB) * np.log(2.0))
            for _ in range(2):
                nc.scalar.activation(lossb[:], llsum[:],
                                     mybir.ActivationFunctionType.Copy,
                                     bias=bias, scale=-1.0)
            nc.sync.dma_start(loss_d[:, 0].unsqueeze(0), lossb[0:1, :])
    nc.compile()
    return nc


_CACHE = {}


def _get_nc():
    if "nc" not in _CACHE:
        _CACHE["nc"] = build_bass()
    return _CACHE["nc"]


def kernel(y_true, y_pred):
    nc = _get_nc()
    in_maps = make_in_maps(y_true, y_pred)
    res = run_bass_kernel_spmd(nc, in_maps, list(range(NCORES)))
    out = np.concatenate([res.results[c]["loss"] for c in range(NCORES)],
                         axis=0)
    return out.astype(F32)
